# revision 18
# baseline (speedup 1.0000x reference)
"""Trainium2 Bass kernel for nn_Block (dense transformer block).

  out = x + FFN(LN2(x + Attn(LN1(x))))   with causal single-head attention,
  B=4, T=2048, C=H=1024, FF=4096, fp32 reference.

Distribution: 8 NeuronCores = (batch b in 0..3) x (query-half in 0..1).
Each core handles one batch element's keys/values and HALF its query rows
(causally balanced interleaved block split), plus LN2+FFN+residual for those
rows.  No collectives; the per-core programs are IDENTICAL (SPMD) - all
per-core variation is input data.

v3: fp8 (e4m3, weights pre-scaled x32 host-side) DoubleRow matmuls for
q/k/v/scores/FFN (~1.5-2x PE rate); softmax-weights @ v stays bf16 for
accuracy.  All h/h2 128x128 transposes run on the DMA xbar
(dma_start_transpose, bf16) followed by a DVE convert-copy to fp8 -- the
PE only does matmuls plus the small p^T transposes.  x2 residual is kept
in SBUF (no DRAM roundtrip) and LN2+h2^T is fused into the attention
pipeline per block.  Phase order: global LN/k/v first (dense PE work),
then own-token LN/q overlapped with the attention score/tail pipeline.
Accumulation is always f32 in PSUM; LN stats / softmax / residuals f32.
"""

import sys
import types

import numpy as np

# ---------------------------------------------------------------------------
# antenv.axon_hooks shim: the image's antenv lacks this module and
# run_bass_kernel_spmd imports it under axon when trace=True.
import antenv

if "antenv.axon_hooks" not in sys.modules:
    _mod = types.ModuleType("antenv.axon_hooks")
    _mod._hook = None
    _mod.set_axon_ntff_profile_hook = lambda h: setattr(_mod, "_hook", h)
    _mod.get_axon_ntff_profile_hook = lambda: _mod._hook
    sys.modules["antenv.axon_hooks"] = _mod
    antenv.axon_hooks = _mod

import ml_dtypes

import concourse.bass as bass
import concourse.mybir as mybir
import concourse.tile as tile
from concourse.bass_utils import run_bass_kernel_spmd

F32 = mybir.dt.float32
BF16 = mybir.dt.bfloat16
F8 = mybir.dt.float8e4
DR = mybir.MatmulPerfMode.DoubleRow
AF = mybir.ActivationFunctionType

B, T, C = 4, 2048, 1024
H, FF = 1024, 4096
P = 128
NT = T // P  # 16 token blocks per batch element
NCT = C // P  # 8 contraction tiles
NH = H // P  # 8 head-dim tiles
NF = FF // P  # 32 ff tiles
TOWN = T // 2  # own tokens per core (1024)
NLOC = TOWN // P  # 8 own blocks
EPS = 1e-5
SW = 32.0  # fp8 weight pre-scale
EXP_SCALE = (1.0 / np.sqrt(np.float32(C))) / (SW * SW)  # folded softmax scale
FF2_INV = 1.0 / (SW * SW)  # FFN2 psum unscale
NEG = -1.0e30

# Causally balanced query-block assignment (sum of chunk counts = 20 each).
L_HALF = [
    [0, 2, 4, 6, 9, 11, 13, 15],
    [1, 3, 5, 7, 8, 10, 12, 14],
]
# ceil((i+1)/4) for i in L_HALF[h] - same sequence for both halves.
NCHUNKS = [1, 1, 2, 2, 3, 3, 4, 4]


def _split_multi_waits(nc):
    """walrus here accepts at most ONE sync-wait per instruction; hoist
    extras onto injected same-engine NoOps."""
    for fn in nc.m.functions:
        for blk in fn.blocks:
            new_insts = []
            changed = False
            for inst in blk.instructions:
                si = getattr(inst, "sync_info", None)
                ow = list(si.on_wait) if si is not None and si.on_wait else []
                if len(ow) > 1:
                    for i, cond in enumerate(ow[:-1]):
                        new_insts.append(
                            mybir.InstNoOp(
                                name=f"{inst.name}-wn{i}",
                                engine=inst.engine,
                                ins=[],
                                outs=[],
                                sync_info=mybir.SyncInfo(
                                    on_wait=[cond], on_update=[]
                                ),
                            )
                        )
                    inst.sync_info = mybir.SyncInfo(
                        on_wait=[ow[-1]], on_update=list(si.on_update or [])
                    )
                    changed = True
                new_insts.append(inst)
            if changed:
                blk.instructions = new_insts


def build_nc():
    from contextlib import ExitStack

    nc = bass.Bass()

    x = nc.declare_dram_parameter("x", [T, C], F32, isOutput=False)
    x_own = nc.declare_dram_parameter("x_own", [TOWN, C], F32, isOutput=False)
    wq = nc.declare_dram_parameter("wq", [P, NCT, H], F8, isOutput=False)
    wk = nc.declare_dram_parameter("wk", [P, NCT, H], F8, isOutput=False)
    wv = nc.declare_dram_parameter("wv", [P, NCT, H], BF16, isOutput=False)
    w1 = nc.declare_dram_parameter("w1", [P, NF, NCT, P], F8, isOutput=False)
    w2 = nc.declare_dram_parameter("w2", [P, NF, C], F8, isOutput=False)
    qb = nc.declare_dram_parameter("qb", [H], F32, isOutput=False)  # x SW
    kb = nc.declare_dram_parameter("kb", [H], F32, isOutput=False)  # x SW
    vb = nc.declare_dram_parameter("vb", [H], F32, isOutput=False)  # unscaled
    b1 = nc.declare_dram_parameter("b1", [FF], F32, isOutput=False)  # x SW
    b2 = nc.declare_dram_parameter("b2", [C], F32, isOutput=False)
    ident = nc.declare_dram_parameter("ident", [P, P], BF16, isOutput=False)
    masks = nc.declare_dram_parameter("masks", [NLOC, P, 512], BF16, isOutput=False)
    out = nc.declare_dram_parameter("out", [TOWN, C], F32, isOutput=True)

    with tile.TileContext(nc) as tc, ExitStack() as top:
        cn = top.enter_context(tc.tile_pool(name="cn", bufs=1))
        ps = top.enter_context(tc.tile_pool(name="ps", bufs=1, space="PSUM"))
        ln = top.enter_context(tc.tile_pool(name="ln", bufs=1))

        # critical-path first: x tile 0 for LN1, then transpose identity.
        x0_t = ln.tile([P, C], F32, tag="xt", bufs=3, name="x0t")
        nc.sync.dma_start(x0_t[:], x[0:P, :])
        # small constants (sync queue)
        qb_t = cn.tile([P, NH], F32)
        nc.sync.dma_start(qb_t[:], qb.rearrange("(m p) -> p m", p=P))
        kb_t = cn.tile([P, NH], F32)
        nc.sync.dma_start(kb_t[:], kb.rearrange("(m p) -> p m", p=P))
        b1_t = cn.tile([P, NF], F32)
        nc.sync.dma_start(b1_t[:], b1.rearrange("(m p) -> p m", p=P))
        id_t = cn.tile([P, P], BF16)
        nc.sync.dma_start(id_t[:], ident[:])
        eps_t = cn.tile([P, 1], F32)
        nc.vector.memset(eps_t, EPS)
        vb_b = cn.tile([P, H], BF16)
        nc.gpsimd.dma_start(vb_b[:], vb[None, :].partition_broadcast(P))
        b2_b = cn.tile([P, C], BF16)
        nc.gpsimd.dma_start(b2_b[:], b2[None, :].partition_broadcast(P))

        px = top.enter_context(tc.tile_pool(name="px", bufs=1))
        x2s = px.tile([P, NLOC, C], F32)    # attn residual, SBUF-resident
        h2T = px.tile([P, NCT, TOWN], F8)   # LN2 output transposed

        _ctr = [0]

        def psum(tag, shape=(P, 512), dt=F32, bufs=2):
            _ctr[0] += 1
            return ps.tile(list(shape), dt, tag=tag, bufs=bufs, name=f"ps{_ctr[0]}")

        def layernorm(x_ap, h_t):
            """h_t (bf16) = (x - mean) * rsqrt(var + eps), stats on free dim."""
            xg = x_ap.rearrange("p (s f) -> p s f", f=512)
            stats = ln.tile([P, 2, nc.vector.BN_STATS_DIM], F32, tag="ln_stats")
            for sg in range(2):
                nc.vector.bn_stats(out=stats[:, sg], in_=xg[:, sg])
            mv = ln.tile([P, nc.vector.BN_AGGR_DIM], F32, tag="ln_mv")
            nc.vector.bn_aggr(out=mv[:], in_=stats[:])
            rstd = ln.tile([P, 1], F32, tag="ln_rstd")
            nc.scalar.activation(
                out=rstd[:], in_=mv[:, 1:2], func=AF.Sqrt,
                bias=eps_t[:], scale=1.0,
            )
            nc.vector.reciprocal(out=rstd[:], in_=rstd[:])
            nc.vector.tensor_scalar(
                out=h_t[:], in0=x_ap,
                scalar1=mv[:, 0:1], scalar2=rstd[:],
                op0=mybir.AluOpType.subtract, op1=mybir.AluOpType.mult,
            )

        def xq():
            # xbar transposes live EXCLUSIVELY on the Scalar hwdge queue --
            # mixing DMATranspose with DMACopy descriptors on one queue
            # corrupts transfers (xbar_mode).
            return nc.scalar

        def ln_tile_pipeline(x_ap, dst_ap):
            """LN(x_ap) -> bf16 h -> DMA-xbar transpose -> fp8 dst_ap."""
            h_t = ln.tile([P, C], BF16, tag="ht", bufs=2)
            layernorm(x_ap, h_t)
            trb = ln.tile([P, NCT, P], BF16, tag="trb", bufs=2)
            xq().dma_start_transpose(trb[:], h_t[:])
            nc.vector.tensor_copy(out=dst_ap, in_=trb[:])
            return trb

        with ExitStack() as sABC:
            pqv = sABC.enter_context(tc.tile_pool(name="pqv", bufs=1))
            qTo = pqv.tile([P, NH, TOWN], F8)    # q^T own tokens (local order)
            kT = pqv.tile([P, NH, T], F8)        # k^T all keys
            v_sb = pqv.tile([P, NT, H], BF16)    # v token-major (bf16)
            pac = sABC.enter_context(tc.tile_pool(name="pac", bufs=1))
            hto = pac.tile([P, NCT, TOWN], F8)
            wq_s = pac.tile([P, NCT, H], F8, name="wqs")

            def a_tile(lt):
                x_t = ln.tile([P, C], F32, tag="xt", bufs=3)
                nc.sync.dma_start(x_t[:], x_own[lt * P : (lt + 1) * P, :])
                ln_tile_pipeline(x_t[:], hto[:, :, lt * P : (lt + 1) * P])

            def q_group(g):
                # q^T for own 512-token group g; stationary reused 2x
                for m in range(NH):
                    acc = psum("c", bufs=4)
                    for k2 in range(NCT // 2):
                        nc.tensor.matmul(
                            acc[:],
                            wq_s[:, 2 * k2 : 2 * k2 + 2, m * P : (m + 1) * P],
                            hto[:, 2 * k2 : 2 * k2 + 2, g * 512 : (g + 1) * 512],
                            start=(k2 == 0),
                            stop=(k2 == NCT // 2 - 1),
                            perf_mode=DR,
                        )
                    nc.vector.tensor_scalar_add(
                        out=qTo[:, m, g * 512 : (g + 1) * 512],
                        in0=acc[:],
                        scalar1=qb_t[:, m : m + 1],
                    )

            with ExitStack() as sB:
                pab = sB.enter_context(tc.tile_pool(name="pab", bufs=1))
                # rolling h^T buffer: halves ping-pong between chunk pairs
                # (0,1) then (2,3); k consumes each pair before reuse.
                hT = pab.tile([P, NCT, 2, 512], F8)
                # projection weights on the GpSimd DMA queue
                wv_s = pab.tile([P, NCT, H], BF16, name="wvs")
                nc.gpsimd.dma_start(wv_s[:], wv[:])
                wk_s = pab.tile([P, NCT, H], F8, name="wks")
                nc.gpsimd.dma_start(wk_s[:], wk[:])
                nc.gpsimd.dma_start(wq_s[:], wq[:])

                def k_chunks(chs):
                    # k^T for 512-key chunks chs; stationary reused len(chs)x
                    for m in range(NH):
                        accs = {ch: psum("c", bufs=4) for ch in chs}
                        for k2 in range(NCT // 2):
                            for ch in chs:
                                nc.tensor.matmul(
                                    accs[ch][:],
                                    wk_s[:, 2 * k2 : 2 * k2 + 2, m * P : (m + 1) * P],
                                    hT[:, 2 * k2 : 2 * k2 + 2, ch % 2, :],
                                    start=(k2 == 0),
                                    stop=(k2 == NCT // 2 - 1),
                                    perf_mode=DR,
                                )
                        for ch in chs:
                            nc.vector.tensor_scalar_add(
                                out=kT[:, m, ch * 512 : (ch + 1) * 512],
                                in0=accs[ch][:],
                                scalar1=kb_t[:, m : m + 1],
                            )

                # ======== Phase B: full LN1 -> hT; v per tile; k ============
                # own-token LN (phase A) tiles 0..3 interleave into late B.
                for st in range(NT):
                    if st == 0:
                        x_t = x0_t
                    else:
                        x_t = ln.tile([P, C], F32, tag="xt", bufs=3)
                        nc.sync.dma_start(x_t[:], x[st * P : (st + 1) * P, :])
                    half = (st // 4) % 2
                    trb = ln_tile_pipeline(
                        x_t[:], hT[:, :, half, (st % 4) * P : (st % 4 + 1) * P]
                    )
                    # v row-block from bf16 h^T (accuracy: no fp8 on v path);
                    # stationary trb block reused 2x
                    va = [psum("a", bufs=2) for _ in range(2)]
                    for k2 in range(NCT):
                        for hh in range(2):
                            nc.tensor.matmul(
                                va[hh][:],
                                trb[:, k2, :],
                                wv_s[:, k2, hh * 512 : (hh + 1) * 512],
                                start=(k2 == 0),
                                stop=(k2 == NCT - 1),
                            )
                    for hh in range(2):
                        nc.vector.tensor_copy(
                            out=v_sb[:, st, hh * 512 : (hh + 1) * 512], in_=va[hh][:]
                        )
                    if st == 7:
                        k_chunks((0, 1))
                    elif st == 15:
                        k_chunks((2, 3))
                    if st >= 8 and st % 2 == 0:
                        a_tile((st - 8) // 2)

            # ======== Phase A + C: own LN1 -> q^T; attention ===============
            with ExitStack() as sC:
                att = sC.enter_context(tc.tile_pool(name="att", bufs=2))
                wtl = sC.enter_context(tc.tile_pool(name="wtl", bufs=2))
                state = {}

                def emit_scores(lp):
                    nch = NCHUNKS[lp]
                    mask_t = att.tile([P, 512], BF16, tag="mask", bufs=3)
                    nc.sync.dma_start(mask_t[:], masks[lp])
                    # prefetch own-x for the tail + fold in vb
                    x_t = att.tile([P, C], F32, tag="xo", bufs=3)
                    nc.sync.dma_start(x_t[:], x_own[lp * P : (lp + 1) * P, :])
                    nc.vector.tensor_add(out=x_t[:], in0=x_t[:], in1=vb_b[:])
                    p_t = att.tile([P, T], BF16, tag="pt", bufs=3)
                    den = att.tile([P, 4], F32, tag="den", bufs=4)
                    scs = [psum("c", bufs=4) for _ in range(nch)]
                    for m2 in range(NH // 2):
                        for j in range(nch):
                            nc.tensor.matmul(
                                scs[j][:],
                                qTo[:, 2 * m2 : 2 * m2 + 2, lp * P : (lp + 1) * P],
                                kT[:, 2 * m2 : 2 * m2 + 2, j * 512 : (j + 1) * 512],
                                start=(m2 == 0),
                                stop=(m2 == NH // 2 - 1),
                                perf_mode=DR,
                            )
                    for j in range(nch):
                        if j == nch - 1:
                            nc.vector.tensor_add(
                                out=scs[j][:], in0=scs[j][:], in1=mask_t[:]
                            )
                        nc.scalar.activation(
                            out=p_t[:, j * 512 : (j + 1) * 512],
                            in_=scs[j][:], func=AF.Exp,
                            scale=float(EXP_SCALE),
                            accum_out=den[:, j : j + 1],
                        )
                    state[lp] = (p_t, den, x_t)

                def emit_tail(lp):
                    nch = NCHUNKS[lp]
                    nst = 4 * nch
                    p_t, den, xvb = state.pop(lp)
                    dsum = att.tile([P, 1], F32, tag="dsum")
                    nc.vector.reduce_sum(
                        out=dsum[:], in_=den[:, :nch], axis=mybir.AxisListType.X
                    )
                    nc.vector.reciprocal(out=dsum[:], in_=dsum[:])
                    # p^T via PE transposes (bf16)
                    wtT = wtl.tile([P, 16, P], BF16, tag="wt")
                    for tg in range((nst + 3) // 4):
                        n4 = min(4, nst - tg * 4)
                        tp = psum("b", (P, 4 * P), BF16)
                        for i in range(n4):
                            stp = tg * 4 + i
                            nc.tensor.transpose(
                                tp[:, i * P : (i + 1) * P],
                                p_t[:, stp * P : (stp + 1) * P],
                                id_t[:],
                            )
                        nc.vector.tensor_copy(
                            out=wtT[:, tg * 4 : tg * 4 + n4, :],
                            in_=tp[:].rearrange("p (s f) -> p s f", f=P)[:, :n4],
                        )
                    sa0 = psum("a", bufs=2)
                    sa1 = psum("a", bufs=2)
                    for stp in range(nst):
                        nc.tensor.matmul(
                            sa0[:], wtT[:, stp, :], v_sb[:, stp, 0:512],
                            start=(stp == 0), stop=(stp == nst - 1),
                        )
                        nc.tensor.matmul(
                            sa1[:], wtT[:, stp, :], v_sb[:, stp, 512:1024],
                            start=(stp == 0), stop=(stp == nst - 1),
                        )
                    for cc, sa in ((0, sa0), (1, sa1)):
                        nc.vector.scalar_tensor_tensor(
                            out=x2s[:, lp, cc * 512 : (cc + 1) * 512],
                            in0=sa[:],
                            scalar=dsum[:],
                            in1=xvb[:, cc * 512 : (cc + 1) * 512],
                            op0=mybir.AluOpType.mult,
                            op1=mybir.AluOpType.add,
                        )
                    # fused LN2 + h2^T for this block
                    ln_tile_pipeline(
                        x2s[:, lp, :], h2T[:, :, lp * P : (lp + 1) * P]
                    )

                q_group(0)
                emit_scores(0)
                emit_scores(1)
                a_tile(4)
                emit_scores(2)
                emit_tail(0)
                a_tile(5)
                emit_scores(3)
                emit_tail(1)
                a_tile(6)
                a_tile(7)
                q_group(1)
                for lp in range(4, NLOC):
                    emit_scores(lp)
                    emit_tail(lp - 2)
                emit_tail(NLOC - 2)
                emit_tail(NLOC - 1)

        # ================= Phase D: FFN (fp8 DR) ===========================
        with ExitStack() as sD:
            big_d = sD.enter_context(tc.tile_pool(name="bigd", bufs=1))
            ffw = sD.enter_context(tc.tile_pool(name="ffw", bufs=3))
            aT = big_d.tile([P, NF, TOWN], F8)
            w2_s = big_d.tile([P, NF, C], F8, name="w2s")

            # a^T = SW*relu(W1^T h2^T + b1); single w1 stream, both halves
            for ft in range(NF):
                w1_t = ffw.tile([P, NCT, P], F8, tag="w1t", bufs=4,
                                name=f"w1t{ft}")
                nc.gpsimd.dma_start(w1_t[:], w1[:, ft])
                accs = [psum("c", bufs=4) for _ in range(2)]
                for k2 in range(NCT // 2):
                    for tch in range(2):
                        nc.tensor.matmul(
                            accs[tch][:],
                            w1_t[:, 2 * k2 : 2 * k2 + 2, :],
                            h2T[:, 2 * k2 : 2 * k2 + 2, tch * 512 : (tch + 1) * 512],
                            start=(k2 == 0),
                            stop=(k2 == NCT // 2 - 1),
                            perf_mode=DR,
                        )
                for tch in range(2):
                    nc.scalar.activation(
                        out=aT[:, ft, tch * 512 : (tch + 1) * 512],
                        in_=accs[tch][:], func=AF.Relu,
                        bias=b1_t[:, ft : ft + 1], scale=1.0,
                    )

            nc.gpsimd.dma_start(w2_s[:], w2[:])

            for lt in range(NLOC):
                xb = ffw.tile([P, C], F32, tag="xb", name=f"xb{lt}")
                nc.vector.tensor_add(out=xb[:], in0=x2s[:, lt, :], in1=b2_b[:])
                grp = [psum("a", bufs=2) for _ in range(2)]
                for f2 in range(NF // 2):
                    for cc in range(2):
                        nc.tensor.matmul(
                            grp[cc][:],
                            aT[:, 2 * f2 : 2 * f2 + 2, lt * P : (lt + 1) * P],
                            w2_s[:, 2 * f2 : 2 * f2 + 2, cc * 512 : (cc + 1) * 512],
                            start=(f2 == 0),
                            stop=(f2 == NF // 2 - 1),
                            perf_mode=DR,
                        )
                o_t = ffw.tile([P, C], F32, tag="ot", name=f"ot{lt}")
                for cc in range(2):
                    nc.vector.scalar_tensor_tensor(
                        out=o_t[:, cc * 512 : (cc + 1) * 512],
                        in0=grp[cc][:],
                        scalar=FF2_INV,
                        in1=xb[:, cc * 512 : (cc + 1) * 512],
                        op0=mybir.AluOpType.mult,
                        op1=mybir.AluOpType.add,
                    )
                nc.sync.dma_start(out[lt * P : (lt + 1) * P, :], o_t[:])

    _split_multi_waits(nc)
    return nc


_NC_CACHE = None


def _get_nc():
    global _NC_CACHE
    if _NC_CACHE is None:
        _NC_CACHE = build_nc()
    return _NC_CACHE


def _to_f8(a):
    return np.ascontiguousarray(
        np.clip(a, -240.0, 240.0).astype(ml_dtypes.float8_e4m3)
    )


def _prep_host(inputs):
    """Fold LN gains/biases into weights; scale weights x32 for fp8;
    build per-core input maps."""
    x = np.asarray(inputs["x"], dtype=np.float32)
    Wk = np.asarray(inputs["Wk"], dtype=np.float32)
    Wq = np.asarray(inputs["Wq"], dtype=np.float32)
    Wv = np.asarray(inputs["Wv"], dtype=np.float32)
    W1 = np.asarray(inputs["W1"], dtype=np.float32)
    b1 = np.asarray(inputs["b1"], dtype=np.float32)
    W2 = np.asarray(inputs["W2"], dtype=np.float32)
    b2 = np.asarray(inputs["b2"], dtype=np.float32)
    g1 = np.asarray(inputs["g1"], dtype=np.float32)
    be1 = np.asarray(inputs["be1"], dtype=np.float32)
    g2 = np.asarray(inputs["g2"], dtype=np.float32)
    be2 = np.asarray(inputs["be2"], dtype=np.float32)

    bf = ml_dtypes.bfloat16
    # [C, H] -> [P, NCT, H] (partition-major contraction tiles)
    def wtile(w):
        return np.ascontiguousarray(w.reshape(NCT, P, H).transpose(1, 0, 2))

    wq_f = _to_f8(wtile(SW * (g1[:, None] * Wq)))
    wk_f = _to_f8(wtile(SW * (g1[:, None] * Wk)))
    wv_f = np.ascontiguousarray(wtile(g1[:, None] * Wv).astype(bf))
    qb = SW * (be1 @ Wq)
    kb = SW * (be1 @ Wk)
    vb = be1 @ Wv
    w1_full = SW * (g2[:, None] * W1)
    w1_f = _to_f8(w1_full.reshape(NCT, P, NF, P).transpose(1, 2, 0, 3))
    w2_f = _to_f8((SW * W2).reshape(NF, P, C).transpose(1, 0, 2))
    b1_f = SW * (b1 + be2 @ W1)

    ident = np.eye(P, dtype=bf)

    # per-half masks: for local position p with global block i, the diagonal
    # 512-key chunk mask is 0 where key-col j <= (i%4)*128 + row else -1e30.
    jj = np.arange(512)[None, :]
    rr = np.arange(P)[:, None]
    masks_h = []
    for half in range(2):
        mk = np.empty((NLOC, P, 512), dtype=bf)
        for ppos, i in enumerate(L_HALF[half]):
            m = i % 4
            mk[ppos] = np.where(jj <= m * P + rr, 0.0, NEG).astype(bf)
        masks_h.append(mk)

    shared = {
        "wq": wq_f, "wk": wk_f, "wv": wv_f, "w1": w1_f, "w2": w2_f,
        "qb": qb, "kb": kb, "vb": vb, "b1": b1_f, "b2": b2,
        "ident": ident,
    }
    in_maps = []
    for core in range(8):
        b, half = core // 2, core % 2
        L = L_HALF[half]
        rows = np.concatenate([np.arange(i * P, (i + 1) * P) for i in L])
        m = dict(shared)
        m["x"] = np.ascontiguousarray(x[b])
        m["x_own"] = np.ascontiguousarray(x[b][rows])
        m["masks"] = masks_h[half]
        in_maps.append(m)
    return in_maps


def _scatter_out(results):
    out = np.empty((B, T, C), dtype=np.float32)
    for core in range(8):
        b, half = core // 2, core % 2
        L = L_HALF[half]
        o = results[core]["out"]
        for ppos, i in enumerate(L):
            out[b, i * P : (i + 1) * P, :] = o[ppos * P : (ppos + 1) * P, :]
    return out


def run(inputs, trace=False, **kw):
    nc = _get_nc()
    in_maps = _prep_host(inputs)
    res = run_bass_kernel_spmd(
        nc, in_maps, core_ids=list(range(8)), trace=trace, **kw
    )
    return _scatter_out(res.results), res


def kernel(**inputs) -> np.ndarray:
    out, _ = run(inputs, trace=False)
    return out
